# revision 13
# baseline (speedup 1.0000x reference)
"""Trainium2 Bass kernel for nn_CriterionPairWiseforWholeFeatAfterPool.

Computation (reference): select feat_ind slice -> MaxPool2d with kernel
(H/2, W/2) producing a 2x2 pooled map per (sample, channel) -> L2-normalize
over channels -> per-sample 4x4 gram over the pooled spatial positions ->
scalar MSE-style loss between teacher/student grams.

Strategy (data-parallel, per the sharding hint): shard the batch axis B=16
across 8 NeuronCores (2 samples/core).  Each core streams its two feature
shards (2 samples x 256 ch x 128 x 128 f32 = 64 MiB) HBM->SBUF with
channels on partitions and reduces every 64x64 max-pool window on the
vector engine (free-axis reduce_max over a strided quadrant view).  Each
core emits its pooled (partial-max) features; the tiny epilogue (fold
partials, per-sample 4x4 gram, normalization, final sum == the all-reduce
of per-core partials) runs on host in a few microseconds of numpy.

Chunking: every pooling band streams as 4 slim 16-row chunks (1 MiB DMA
ops, 8 KiB contiguous per partition -> 128 descriptors, perfectly flat
across the 16 SDMA engines).  Slim chunks keep the vector engine's reduce
granularity small, so it tracks the stream with <=1-chunk (~2.4 us) lag:
bytes start flowing ~2 us after block entry and the final reduce lands
~2.5 us after the last byte.  NBUF=18 slots (18 MiB of stream in flight)
absorb the DVE's mid-stream jitter so the DMA queue never stalls on slot
reuse (the old 12-slot version lost ~9 us to those stalls).  The block
skips the gpsimd dge-drain at exit (gpsimd unused).

Known hardware asymmetry (measured, not fixable in-kernel): on even
physical NeuronCores one edge SDMA engine (idx 0 or 15) runs ~20% slower
when the pair-sharing core is also streaming; HWDGE splits every op's
descriptors positionally (engine = position % 16 from 0), so no op shape
can give the edge engines a smaller byte share without starving the
middle engines first — those cores are bound at ~195-215 us while the
clean cores finish in ~170 us.
"""

import contextlib

import numpy as np

import concourse.bacc as bacc
import concourse.mybir as mybir
from concourse.bass_utils import run_bass_kernel_spmd

N_CORES = 8
P = 128           # SBUF partitions
B_LOC = 2         # samples per core (16 / 8)
C = 256           # channels
H = 128
W = 128
BAND = 64         # pooling-window rows
SPLIT = 4         # chunks per band (16 rows, 1 MiB each)
ROWS = BAND // SPLIT
FREE = ROWS * W   # f32 elements per partition per chunk (8 KiB)
NBUF = 18         # SBUF slots (18 x 8 KiB/partition = 144 KiB)

N_BANDS = B_LOC * 2 * (C // P) * (H // BAND)   # 16 bands
N_XFERS = N_BANDS * SPLIT                      # 64 slim chunks
N_COLS = N_XFERS * 2                           # pooled cols (pairs)

_NC = None


def _xfer_meta():
    """Transfer list metadata: (band_index, sub_row_offset, rows).

    Chunk order interleaves the S and T streams (x innermost): the core
    walks two sequential address streams instead of one, which spreads the
    pair's HBM bank traffic when both cores of an HBM domain stream."""
    metas = []
    for b in range(B_LOC):
        for cb in range(C // P):
            for band in range(H // BAND):
                for k in range(SPLIT):
                    for x in range(2):
                        bi = ((b * 2 + x) * (C // P) + cb) * (H // BAND) + band
                        metas.append((bi, k * ROWS, ROWS))
    return metas


def _band_addr(bi):
    """band index -> (x_idx, b, cb, band)."""
    band = bi % (H // BAND)
    r = bi // (H // BAND)
    cb = r % (C // P)
    r //= C // P
    x = r % 2
    b = r // 2
    return x, b, cb, band


def _build_nc():
    """Build + compile the per-core SPMD Bass program (same NEFF on all cores)."""
    nc = bacc.Bacc("TRN2", target_bir_lowering=False, debug=False,
                   num_devices=N_CORES)
    s = nc.dram_tensor("s", [B_LOC, C, H, W], mybir.dt.float32,
                       kind="ExternalInput").ap()
    t = nc.dram_tensor("t", [B_LOC, C, H, W], mybir.dt.float32,
                       kind="ExternalInput").ap()
    out = nc.dram_tensor("pooled", [P, N_COLS], mybir.dt.float32,
                         kind="ExternalOutput").ap()

    # transfer list: (2-D dram source AP, free elems, rows covered)
    xfers = []
    for bi, r_off, rows in _xfer_meta():
        xi, b, cb, band = _band_addr(bi)
        x = (s, t)[xi]
        r0 = band * BAND + r_off
        src = x[b, cb * P:(cb + 1) * P, r0:r0 + rows, :]
        xfers.append((src.rearrange("c h w -> c (h w)"), rows * W, rows))
    n = len(xfers)
    assert n == N_XFERS

    with contextlib.ExitStack() as ctx:
        bufs = [ctx.enter_context(
            nc.sbuf_tensor(f"buf{i}", [P, FREE], mybir.dt.float32))
            for i in range(NBUF)]
        pooled = ctx.enter_context(
            nc.sbuf_tensor("pooled_sb", [P, N_COLS], mybir.dt.float32))
        # one DMA-completion semaphore per buffer slot: at most one in-flight
        # DMA per semaphore (slot reuse is serialized through red_sem), so
        # concurrent DMAs never race on the same semaphore
        dma_sems = [ctx.enter_context(nc.semaphore(f"dma_sem{i}"))
                    for i in range(NBUF)]
        out_sem = ctx.enter_context(nc.semaphore("out_sem"))
        red_sem = ctx.enter_context(nc.semaphore("red_sem"))
        # gpsimd is unused; skip its expensive dge_drain at block exit
        block = ctx.enter_context(nc.Block(no_gpsimd_drain=True))

        @block.sync
        def _(sync):
            for i, (src, free, _h) in enumerate(xfers):
                if i >= NBUF:
                    # slot reuse: wait until the reduce of tile i-NBUF is done
                    sync.wait_ge(red_sem, i - NBUF + 1)
                sync.dma_start(
                    bufs[i % NBUF][:, :free], src).then_inc(
                        dma_sems[i % NBUF], 16)
            # split the pooled write-back: the bulk goes out while the last
            # chunks are still reducing, so only a 2-column DMA and its
            # completion receipt sit after the final reduce
            sync.wait_ge(red_sem, n - 1)
            sync.dma_start(out[:, :2 * (n - 1)],
                           pooled[:, :2 * (n - 1)]).then_inc(out_sem, 16)
            sync.wait_ge(red_sem, n)
            sync.dma_start(out[:, 2 * (n - 1):],
                           pooled[:, 2 * (n - 1):]).then_inc(out_sem, 16)
            sync.wait_ge(out_sem, 32)

        @block.vector
        def _(vector):
            for i, (_src, free, h) in enumerate(xfers):
                vector.wait_ge(dma_sems[i % NBUF], 16 * (i // NBUF + 1))
                # free dim is (h, w) row-major; expose the two 64-wide halves
                # as an outer axis, reduce the h x 64 window per half
                view = bufs[i % NBUF][:, :free].rearrange(
                    "c (h j w) -> c j h w", h=h, j=2, w=64)
                vector.tensor_reduce(
                    pooled[:, 2 * i:2 * i + 2], view,
                    axis=mybir.AxisListType.XY,
                    op=mybir.AluOpType.max).then_inc(red_sem, 1)

    nc.compile()
    return nc


def get_nc():
    global _NC
    if _NC is None:
        _NC = _build_nc()
    return _NC


def make_in_maps(fS, fT):
    """Per-core input dicts: batch-sharded contiguous slices."""
    return [{"s": np.ascontiguousarray(fS[B_LOC * i:B_LOC * (i + 1)]),
             "t": np.ascontiguousarray(fT[B_LOC * i:B_LOC * (i + 1)])}
            for i in range(N_CORES)]


def finish(pooled_list):
    """Host epilogue: reassemble pooled features, gram + normalize + loss."""
    B = B_LOC * N_CORES
    fS = np.full((B, C, 4), -np.inf)
    fT = np.full((B, C, 4), -np.inf)
    metas = _xfer_meta()
    for i, arr in enumerate(pooled_list):
        a = np.asarray(arr)  # [P, N_COLS]; cols 2k,2k+1 = quadrant pair
        f = (fS, fT)
        for k, (bi, _r_off, _rows) in enumerate(metas):
            xi, bl, cb, band = _band_addr(bi)
            tgt = f[xi][i * B_LOC + bl, cb * P:(cb + 1) * P,
                        band * 2:band * 2 + 2]
            np.maximum(tgt, a[:, 2 * k:2 * k + 2], out=tgt)

    def sim(f):
        G = np.einsum('bcm,bcn->bmn', f, f)
        d = np.sqrt(np.einsum('bmm->bm', G)) + 1e-8
        return G / (d[:, :, None] * d[:, None, :])

    loss = ((sim(fT) - sim(fS)) ** 2).sum() / (4 * 4) / B
    return np.float32(loss)


def run_device(fS, fT, **spmd_kwargs):
    """Run the compiled program on the 8 cores; returns (pooled_list, results)."""
    res = run_bass_kernel_spmd(get_nc(), make_in_maps(fS, fT),
                               core_ids=list(range(N_CORES)), **spmd_kwargs)
    pooled_list = [res.results[i]["pooled"] for i in range(N_CORES)]
    return pooled_list, res


def kernel(preds_S, preds_T, feat_ind):
    fi = int(np.asarray(feat_ind))
    fS = np.ascontiguousarray(np.asarray(preds_S)[fi], dtype=np.float32)
    fT = np.ascontiguousarray(np.asarray(preds_T)[fi], dtype=np.float32)
    try:
        pooled_list, _ = run_device(fS, fT)
    except Exception:
        # one retry: a cold device occasionally reports a transient
        # NRT execution error on the very first NEFF launch
        pooled_list, _ = run_device(fS, fT)
    return finish(pooled_list)
